# revision 8
# baseline (speedup 1.0000x reference)
"""Multi-head self-attention (B=2, S=2048, D=1024, H=16) on 8 TRN2 cores.

Sharding: tensor-parallel over heads. Core c owns heads 2c, 2c+1:
  - wq/wk/wv column slice [:, 128c:128(c+1)], wo row slice [128c:128(c+1), :]
  - every core gets the full input x; returns a partial output (row-parallel
    wo partial sum); host reduces the 8 partials and adds bo.

Per-core kernel (all matmuls in fp32r = full-rate rounded fp32):
  phase 1: X^T is pre-transposed on the host; DMA X^T s-chunks and
           compute Q^T/K^T/V^T = W^T X^T (+bias).
  phase 2: per (batch, head): V natural via PE transpose; then per q-chunk:
           S^T = K^T_tile^T Q^T (k on partitions), P^T = exp(0.125 S^T) on ACT,
           AV via lhsT=[V|ones] giving O^T rows + rowsum row for free,
           normalize via E-matmul broadcast of reciprocal rowsums,
           output projection from O^T, DMA partial rows out.
"""
import numpy as np
import concourse.bass as bass
import concourse.mybir as mybir
import concourse.tile as tile
from concourse import bacc
from concourse.bass import ts, ds
from concourse.masks import make_identity

F32 = mybir.dt.float32
F32R = mybir.dt.float32r
EXP = mybir.ActivationFunctionType.Exp

B = 2
D = 1024
HD = 128   # head-dims per core (2 heads x 64)
SEQ = 2048
NCORES = 8


def _emit(nc, tc, seq, qcw):
    NJ = seq // 512          # phase-1 s-chunks per batch
    NKT = seq // 128         # k-tiles per batch
    NQC = seq // qcw         # attention q-chunks per batch
    NQS = qcw // 128         # outproj q-subtiles per q-chunk
    NNH = qcw // 512         # 512-wide matmul slices per q-chunk
    bs = B * seq

    xt_d = nc.dram_tensor("xt", [D, bs], F32, kind="ExternalInput")
    wq_d = nc.dram_tensor("wq", [D, HD], F32, kind="ExternalInput")
    wk_d = nc.dram_tensor("wk", [D, HD], F32, kind="ExternalInput")
    wv_d = nc.dram_tensor("wv", [D, HD], F32, kind="ExternalInput")
    bq_d = nc.dram_tensor("bq", [HD, 1], F32, kind="ExternalInput")
    bk_d = nc.dram_tensor("bk", [HD, 1], F32, kind="ExternalInput")
    bv_d = nc.dram_tensor("bv", [HD, 1], F32, kind="ExternalInput")
    wo_d = nc.dram_tensor("wo", [HD, D], F32, kind="ExternalInput")
    out = nc.dram_tensor("out", [bs, D], F32, kind="ExternalOutput")

    with (
        nc.allow_low_precision(reason="fp32r matmul pipeline is intentional"),
        tc.tile_pool(name="wp", bufs=1) as wp,
        tc.tile_pool(name="cp", bufs=1) as cp,
        tc.tile_pool(name="xtp", bufs=2) as xtp,
        tc.tile_pool(name="qkvp", bufs=2) as qkvp,
        tc.tile_pool(name="vnp", bufs=2) as vnp,
        tc.tile_pool(name="ptp", bufs=4) as ptp,
        tc.tile_pool(name="otp", bufs=2) as otp,
        tc.tile_pool(name="rtsp", bufs=2) as rtsp,
        tc.tile_pool(name="osbp", bufs=3) as osbp,
        tc.tile_pool(name="psp", bufs=2, space="PSUM") as psp,
    ):
        # ---- constants & weights
        WQ = wp.tile([128, D], F32R, tag="WQ")
        WK = wp.tile([128, D], F32R, tag="WK")
        WV = wp.tile([128, D], F32R, tag="WV")
        WO = wp.tile([128, D], F32R, tag="WO")
        for W, w_d in ((WQ, wq_d), (WK, wk_d), (WV, wv_d)):
            nc.sync.dma_start(
                W[:].rearrange("p (k m) -> p k m", m=HD),
                w_d[:, :].bitcast(F32R).rearrange("(k p) m -> p k m", p=128),
            )
        nc.sync.dma_start(WO[:], wo_d[:, :].bitcast(F32R))

        Bq = cp.tile([128, 1], F32, tag="Bq")
        Bk = cp.tile([128, 1], F32, tag="Bk")
        Bv = cp.tile([128, 1], F32, tag="Bv")
        nc.sync.dma_start(Bq[:], bq_d[:, :])
        nc.sync.dma_start(Bk[:], bk_d[:, :])
        nc.sync.dma_start(Bv[:], bv_d[:, :])

        ident = cp.tile([128, 128], F32, tag="ident")
        make_identity(nc, ident[:])
        identr = cp.tile([128, 128], F32R, tag="identr")
        nc.vector.tensor_copy(identr[:], ident[:])

        # E routes reciprocal-rowsum rows to head row-blocks:
        # rt[m,q] = sum_k E[k,m] rec[k,q]; rec row 64 = 1/rowsum_h0 (-> m<64),
        # rec row 0 = 1/rowsum_h1 (-> m>=64).
        E = cp.tile([128, 128], F32R, tag="E")
        nc.gpsimd.memset(E[:].bitcast(F32), 0.0)
        nc.gpsimd.memset(E[:].bitcast(F32)[ds(64, 1), 0:64], 1.0)
        nc.gpsimd.memset(E[:].bitcast(F32)[ds(0, 1), 64:128], 1.0)

        rec = cp.tile([128, qcw], F32R, tag="rec")
        nc.gpsimd.memset(rec[:].bitcast(F32), 0.0)

        for b in range(B):
            # ---- phase 1: projections for batch b
            QT = qkvp.tile([128, seq], F32R, tag="qt")
            KT = qkvp.tile([128, seq], F32R, tag="kt")
            VT = qkvp.tile([128, seq], F32R, tag="vt")
            for j in range(NJ):
                col0 = b * seq + j * 512
                xt = xtp.tile([128, 8 * 512], F32R, tag="xt")
                for kd in range(8):
                    nc.sync.dma_start(
                        xt[:, ts(kd, 512)],
                        xt_d[ds(kd * 128, 128), ds(col0, 512)].bitcast(F32R),
                    )
                for W, Bt, DST in ((WQ, Bq, QT), (WK, Bk, KT), (WV, Bv, VT)):
                    acc = psp.tile([128, 1024], F32, tag="stps")
                    for kd in range(8):
                        nc.tensor.matmul(
                            acc[:, 0:512],
                            W[:, ts(kd, 128)],
                            xt[:, ts(kd, 512)],
                            start=(kd == 0),
                            stop=(kd == 7),
                        )
                    nc.vector.tensor_scalar_add(
                        DST[:, ts(j, 512)], acc[:, 0:512], Bt[:, 0:1]
                    )

            # ---- phase 2a: V in natural layout.
            # h0: vn0 kt-blocks of 96 cols = [V(64) | ones | zeros(31)]
            #     -> av0[0:96]: O^T_h0 rows 0-63, rowsum_h0 row 64.
            # h1: vn1 kt-blocks of 128 cols = [ones | zeros(63) | V(64)]
            #     -> av1: rowsum_h1 row 0, O^T_h1 rows 64-127.
            vn0 = vnp.tile([128, NKT * 96], F32R, tag="vn0")
            vn03 = vn0[:].rearrange("p (k c) -> p k c", c=96)
            nc.gpsimd.memset(vn03.bitcast(F32)[:, :, ds(64, 1)], 1.0)
            nc.gpsimd.memset(vn03.bitcast(F32)[:, :, ds(65, 31)], 0.0)
            vn1 = vnp.tile([128, NKT * 128], F32R, tag="vn1")
            vn13 = vn1[:].rearrange("p (k c) -> p k c", c=128)
            nc.gpsimd.memset(vn13.bitcast(F32)[:, :, ds(0, 1)], 1.0)
            nc.gpsimd.memset(vn13.bitcast(F32)[:, :, ds(1, 63)], 0.0)
            for h in range(2):
                for g in range(NKT // 4):
                    ptr = psp.tile([128, 1024], F32, tag="stps")
                    for i in range(4):
                        kt = g * 4 + i
                        nc.tensor.transpose(
                            ptr[:, ts(i, 64)].bitcast(F32R),
                            VT[ds(64 * h, 64), ds(kt * 128, 128)],
                            identr[ds(64 * h, 64), ds(64 * h, 64)],
                        )
                    if h == 0:
                        nc.vector.tensor_copy(
                            vn03[:, ds(g * 4, 4), ds(0, 64)],
                            ptr[:, 0:256].rearrange("p (k c) -> p k c", c=64),
                        )
                    else:
                        nc.vector.tensor_copy(
                            vn13[:, ds(g * 4, 4), ds(64, 64)],
                            ptr[:, 0:256].rearrange("p (k c) -> p k c", c=64),
                        )

            # ---- phase 2b: attention + output projection
            for qc in range(NQC):
                q0 = b * seq + qc * qcw
                avs = [
                    psp.tile([128, qcw], F32, tag="avps", name=f"av{h}")
                    for h in range(2)
                ]
                for kt in range(NKT):
                    sts = []
                    for h in range(2):
                        stp = psp.tile([128, qcw], F32, tag="stps",
                                       name=f"st{h}")
                        for nh in range(NNH):
                            nc.tensor.matmul(
                                stp[:, ts(nh, 512)],
                                KT[ds(64 * h, 64), ds(kt * 128, 128)],
                                QT[ds(64 * h, 64), ds(qc * qcw + nh * 512, 512)],
                                start=True,
                                stop=True,
                            )
                        sts.append(stp)
                    pts = []
                    for h in range(2):
                        pt = ptp.tile([128, qcw], F32R, tag="pt",
                                      name=f"pt{h}")
                        nc.scalar.activation(pt[:], sts[h][:], EXP, scale=0.125)
                        pts.append(pt)
                    for nh in range(NNH):
                        nc.tensor.matmul(
                            avs[0][0:96, ts(nh, 512)],
                            vn0[:, ds(kt * 96, 96)],
                            pts[0][:, ts(nh, 512)],
                            start=(kt == 0),
                            stop=(kt == NKT - 1),
                        )
                        nc.tensor.matmul(
                            avs[1][:, ts(nh, 512)],
                            vn1[:, ds(kt * 128, 128)],
                            pts[1][:, ts(nh, 512)],
                            start=(kt == 0),
                            stop=(kt == NKT - 1),
                        )
                # normalize: rowsum_h0 at av0 row 64, rowsum_h1 at av1 row 0.
                # ACT copies the raw sum rows into rec; E-matmul broadcasts the
                # sums to head row-blocks; one full-width fast reciprocal.
                nc.scalar.copy(rec[ds(64, 1), :], avs[0][ds(64, 1), :])
                nc.scalar.copy(rec[ds(0, 1), :], avs[1][ds(0, 1), :])
                rt = psp.tile([128, qcw], F32, tag="stps")
                for nh in range(NNH):
                    nc.tensor.matmul(
                        rt[:, ts(nh, 512)], E[:], rec[:, ts(nh, 512)],
                        start=True, stop=True,
                    )
                rts = rtsp.tile([128, qcw], F32, tag="rts")
                nc.vector.reciprocal_approx_fast(rts[:], rt[:])
                ot = otp.tile([128, qcw], F32R, tag="ot")
                nc.vector.tensor_mul(ot[0:64, :], avs[0][0:64, :], rts[0:64, :])
                nc.vector.tensor_mul(
                    ot[ds(64, 64), :], avs[1][ds(64, 64), :], rts[ds(64, 64), :]
                )
                for qs in range(NQS):
                    op = psp.tile([128, 1024], F32, tag="avps")
                    for n2 in range(2):
                        nc.tensor.matmul(
                            op[:, ts(n2, 512)],
                            ot[:, ts(qs, 128)],
                            WO[:, ts(n2, 512)],
                            start=True,
                            stop=True,
                        )
                    osb = osbp.tile([128, D], F32, tag="osb")
                    if qs % 2 == 0:
                        nc.vector.tensor_copy(osb[:], op[:])
                    else:
                        nc.scalar.copy(osb[:], op[:])
                    nc.sync.dma_start(out[ds(q0 + qs * 128, 128), :], osb[:])


def build_nc(seq=SEQ, qcw=1024):
    assert seq % 512 == 0 and qcw % 512 == 0 and seq % qcw == 0
    assert (seq // 128) % 4 == 0
    nc = bacc.Bacc("TRN2", target_bir_lowering=False, debug=False,
                   num_devices=NCORES)
    with tile.TileContext(nc) as tc:
        _emit(nc, tc, seq, qcw)
    nc.compile()
    return nc


def shard_inputs(inputs, wq, bq, wk, bk, wv, bv, wo, seq=SEQ):
    X2 = np.asarray(inputs, np.float32).reshape(B * seq, D)
    XT = np.ascontiguousarray(X2.T)
    wq, wk, wv = (np.asarray(a, np.float32) for a in (wq, wk, wv))
    bq, bk, bv = (np.asarray(a, np.float32) for a in (bq, bk, bv))
    wo = np.asarray(wo, np.float32)
    in_maps = []
    for c in range(NCORES):
        sl = slice(c * HD, (c + 1) * HD)
        in_maps.append({
            "xt": XT,
            "wq": np.ascontiguousarray(wq[:, sl]),
            "wk": np.ascontiguousarray(wk[:, sl]),
            "wv": np.ascontiguousarray(wv[:, sl]),
            "bq": np.ascontiguousarray(bq[sl]).reshape(HD, 1),
            "bk": np.ascontiguousarray(bk[sl]).reshape(HD, 1),
            "bv": np.ascontiguousarray(bv[sl]).reshape(HD, 1),
            "wo": np.ascontiguousarray(wo[sl, :]),
        })
    return in_maps


_NC_CACHE = {}


def _get_nc(seq=SEQ, qcw=1024):
    key = (seq, qcw)
    if key not in _NC_CACHE:
        _NC_CACHE[key] = build_nc(seq, qcw)
    return _NC_CACHE[key]


def kernel(inputs, wq, bq, wk, bk, wv, bv, wo, bo):
    from concourse.bass_utils import run_bass_kernel_spmd

    nc = _get_nc()
    in_maps = shard_inputs(inputs, wq, bq, wk, bk, wv, bv, wo)
    res = run_bass_kernel_spmd(nc, in_maps, core_ids=list(range(NCORES)))
    acc = np.zeros((B * SEQ, D), np.float64)
    for c in range(NCORES):
        acc += res.results[c]["out"].astype(np.float64)
    acc += np.asarray(bo, np.float32).astype(np.float64)
    return acc.astype(np.float32).reshape(B, SEQ, D)


# revision 10
# speedup vs baseline: 1.1949x; 1.1949x over previous
"""Multi-head self-attention (B=2, S=2048, D=1024, H=16) on 8 TRN2 cores.

Sharding: tensor-parallel over heads. Core c owns heads 2c, 2c+1:
  - wq/wk/wv column slice [:, 128c:128(c+1)], wo row slice [128c:128(c+1), :]
  - every core gets the full input x; returns a partial output (row-parallel
    wo partial sum); host reduces the 8 partials and adds bo.

Per-core kernel (all matmuls in fp32r = full-rate rounded fp32):
  phase 1: X^T is pre-transposed on the host; DMA X^T s-chunks and
           compute Q^T/K^T/V^T = W^T X^T (+bias).
  phase 2: per (batch, head): V natural via PE transpose; then per q-chunk:
           S^T = K^T_tile^T Q^T (k on partitions), P^T = exp(0.125 S^T) on ACT,
           AV via lhsT=[V|ones] giving O^T rows + rowsum row for free,
           normalize via E-matmul broadcast of reciprocal rowsums,
           output projection from O^T, DMA partial rows out.
"""
import numpy as np
import concourse.bass as bass
import concourse.mybir as mybir
import concourse.tile as tile
from concourse import bacc
from concourse.bass import ts, ds
from concourse.masks import make_identity

F32 = mybir.dt.float32
F32R = mybir.dt.float32r
EXP = mybir.ActivationFunctionType.Exp

B = 2
D = 1024
HD = 128   # head-dims per core (2 heads x 64)
SEQ = 2048
NCORES = 8


def _emit(nc, tc, seq, qcw):
    NJ = seq // 512          # phase-1 s-chunks per batch
    NKT = seq // 128         # k-tiles per batch
    NQC = seq // qcw         # attention q-chunks per batch
    NQS = qcw // 128         # outproj q-subtiles per q-chunk
    NNH = qcw // 512         # 512-wide matmul slices per q-chunk
    bs = B * seq

    xt_d = nc.dram_tensor("xt", [D, bs], F32, kind="ExternalInput")
    wq_d = nc.dram_tensor("wq", [D, HD], F32, kind="ExternalInput")
    wk_d = nc.dram_tensor("wk", [D, HD], F32, kind="ExternalInput")
    wv_d = nc.dram_tensor("wv", [D, HD], F32, kind="ExternalInput")
    bq_d = nc.dram_tensor("bq", [HD, 1], F32, kind="ExternalInput")
    bk_d = nc.dram_tensor("bk", [HD, 1], F32, kind="ExternalInput")
    bv_d = nc.dram_tensor("bv", [HD, 1], F32, kind="ExternalInput")
    wo_d = nc.dram_tensor("wo", [HD, D], F32, kind="ExternalInput")
    out = nc.dram_tensor("out", [bs, D], F32, kind="ExternalOutput")

    with (
        nc.allow_low_precision(reason="fp32r matmul pipeline is intentional"),
        tc.tile_pool(name="wp", bufs=1) as wp,
        tc.tile_pool(name="cp", bufs=1) as cp,
        tc.tile_pool(name="xtp", bufs=2) as xtp,
        tc.tile_pool(name="qkvp", bufs=2) as qkvp,
        tc.tile_pool(name="vnp", bufs=4) as vnp,
        tc.tile_pool(name="ptp", bufs=5) as ptp,
        tc.tile_pool(name="otp", bufs=2) as otp,
        tc.tile_pool(name="rtsp", bufs=2) as rtsp,
        tc.tile_pool(name="osbp", bufs=3) as osbp,
        tc.tile_pool(name="psp", bufs=2, space="PSUM") as psp,
    ):
        # ---- constants & weights
        WQ = wp.tile([128, D], F32R, tag="WQ")
        WK = wp.tile([128, D], F32R, tag="WK")
        WV = wp.tile([128, D], F32R, tag="WV")
        WO = wp.tile([128, D], F32R, tag="WO")
        for W, w_d in ((WQ, wq_d), (WK, wk_d), (WV, wv_d)):
            nc.sync.dma_start(
                W[:].rearrange("p (k m) -> p k m", m=HD),
                w_d[:, :].bitcast(F32R).rearrange("(k p) m -> p k m", p=128),
            )
        nc.sync.dma_start(WO[:], wo_d[:, :].bitcast(F32R))

        Bq = cp.tile([128, 1], F32, tag="Bq")
        Bk = cp.tile([128, 1], F32, tag="Bk")
        Bv = cp.tile([128, 1], F32, tag="Bv")
        nc.sync.dma_start(Bq[:], bq_d[:, :])
        nc.sync.dma_start(Bk[:], bk_d[:, :])
        nc.sync.dma_start(Bv[:], bv_d[:, :])

        ident = cp.tile([128, 128], F32, tag="ident")
        make_identity(nc, ident[:])
        identr = cp.tile([128, 128], F32R, tag="identr")
        nc.vector.tensor_copy(identr[:], ident[:])

        # E routes reciprocal-rowsum rows to head row-blocks:
        # rt[m,q] = sum_k E[k,m] rec[k,q]; rec row 64 = 1/rowsum_h0 (-> m<64),
        # rec row 0 = 1/rowsum_h1 (-> m>=64).
        E = cp.tile([128, 128], F32R, tag="E")
        nc.gpsimd.memset(E[:].bitcast(F32), 0.0)
        nc.gpsimd.memset(E[:].bitcast(F32)[ds(64, 1), 0:64], 1.0)
        nc.gpsimd.memset(E[:].bitcast(F32)[ds(0, 1), 64:128], 1.0)

        rec = cp.tile([128, qcw], F32R, tag="rec")
        nc.gpsimd.memset(rec[:].bitcast(F32), 0.0)

        for b in range(B):
            # ---- phase 1: projections for batch b
            QT = qkvp.tile([128, seq], F32R, tag="qt")
            KT = qkvp.tile([128, seq], F32R, tag="kt")
            VT = qkvp.tile([128, seq], F32R, tag="vt")
            for j in range(NJ):
                col0 = b * seq + j * 512
                xt = xtp.tile([128, 8 * 512], F32R, tag="xt")
                for kd in range(8):
                    nc.sync.dma_start(
                        xt[:, ts(kd, 512)],
                        xt_d[ds(kd * 128, 128), ds(col0, 512)].bitcast(F32R),
                    )
                for W, Bt, DST in ((WQ, Bq, QT), (WK, Bk, KT), (WV, Bv, VT)):
                    acc = psp.tile([128, 1024], F32, tag="stps")
                    for kd in range(8):
                        nc.tensor.matmul(
                            acc[:, 0:512],
                            W[:, ts(kd, 128)],
                            xt[:, ts(kd, 512)],
                            start=(kd == 0),
                            stop=(kd == 7),
                        )
                    nc.vector.tensor_scalar_add(
                        DST[:, ts(j, 512)], acc[:, 0:512], Bt[:, 0:1]
                    )

            # ---- phase 2a: V in natural layout.
            # h0: vn0 kt-blocks of 96 cols = [V(64) | ones | zeros(31)]
            #     -> av0[0:96]: O^T_h0 rows 0-63, rowsum_h0 row 64.
            # h1: vn1 kt-blocks of 128 cols = [ones | zeros(63) | V(64)]
            #     -> av1: rowsum_h1 row 0, O^T_h1 rows 64-127.
            vn0 = vnp.tile([128, NKT * 96], F32R, tag="vn0")
            vn03 = vn0[:].rearrange("p (k c) -> p k c", c=96)
            nc.gpsimd.memset(vn03.bitcast(F32)[:, :, ds(64, 1)], 1.0)
            nc.gpsimd.memset(vn03.bitcast(F32)[:, :, ds(65, 31)], 0.0)
            vn1 = vnp.tile([128, NKT * 128], F32R, tag="vn1")
            vn13 = vn1[:].rearrange("p (k c) -> p k c", c=128)
            nc.gpsimd.memset(vn13.bitcast(F32)[:, :, ds(0, 1)], 1.0)
            nc.gpsimd.memset(vn13.bitcast(F32)[:, :, ds(1, 63)], 0.0)
            for h in range(2):
                for g in range(NKT // 4):
                    ptr = psp.tile([128, 1024], F32, tag="stps")
                    for i in range(4):
                        kt = g * 4 + i
                        nc.tensor.transpose(
                            ptr[:, ts(i, 64)].bitcast(F32R),
                            VT[ds(64 * h, 64), ds(kt * 128, 128)],
                            identr[ds(64 * h, 64), ds(64 * h, 64)],
                        )
                    if h == 0:
                        nc.vector.tensor_copy(
                            vn03[:, ds(g * 4, 4), ds(0, 64)],
                            ptr[:, 0:256].rearrange("p (k c) -> p k c", c=64),
                        )
                    else:
                        nc.vector.tensor_copy(
                            vn13[:, ds(g * 4, 4), ds(64, 64)],
                            ptr[:, 0:256].rearrange("p (k c) -> p k c", c=64),
                        )

            # ---- phase 2b: attention + output projection
            for qc in range(NQC):
                q0 = b * seq + qc * qcw
                avs = [
                    psp.tile([128, qcw], F32, tag="avps", name=f"av{h}")
                    for h in range(2)
                ]
                for kt in range(NKT):
                    sts = []
                    for h in range(2):
                        stp = psp.tile([128, qcw], F32, tag="stps",
                                       name=f"st{h}")
                        for nh in range(NNH):
                            nc.tensor.matmul(
                                stp[:, ts(nh, 512)],
                                KT[ds(64 * h, 64), ds(kt * 128, 128)],
                                QT[ds(64 * h, 64), ds(qc * qcw + nh * 512, 512)],
                                start=True,
                                stop=True,
                            )
                        sts.append(stp)
                    pts = []
                    for h in range(2):
                        pt = ptp.tile([128, qcw], F32R, tag="pt",
                                      name=f"pt{h}")
                        nc.scalar.activation(pt[:], sts[h][:], EXP, scale=0.125)
                        pts.append(pt)
                    for nh in range(NNH):
                        nc.tensor.matmul(
                            avs[0][0:96, ts(nh, 512)],
                            vn0[:, ds(kt * 96, 96)],
                            pts[0][:, ts(nh, 512)],
                            start=(kt == 0),
                            stop=(kt == NKT - 1),
                        )
                        nc.tensor.matmul(
                            avs[1][:, ts(nh, 512)],
                            vn1[:, ds(kt * 128, 128)],
                            pts[1][:, ts(nh, 512)],
                            start=(kt == 0),
                            stop=(kt == NKT - 1),
                        )
                # normalize: rowsum_h0 at av0 row 64, rowsum_h1 at av1 row 0.
                # ACT copies the raw sum rows into rec; E-matmul broadcasts the
                # sums to head row-blocks; one full-width fast reciprocal.
                nc.scalar.copy(rec[ds(64, 1), :], avs[0][ds(64, 1), :])
                nc.scalar.copy(rec[ds(0, 1), :], avs[1][ds(0, 1), :])
                rt = psp.tile([128, qcw], F32, tag="stps")
                for nh in range(NNH):
                    nc.tensor.matmul(
                        rt[:, ts(nh, 512)], E[:], rec[:, ts(nh, 512)],
                        start=True, stop=True,
                    )
                rts = rtsp.tile([128, qcw], F32, tag="rts")
                nc.vector.reciprocal_approx_fast(rts[:], rt[:])
                ot = otp.tile([128, qcw], F32R, tag="ot")
                nc.vector.tensor_mul(ot[0:64, :], avs[0][0:64, :], rts[0:64, :])
                nc.vector.tensor_mul(
                    ot[ds(64, 64), :], avs[1][ds(64, 64), :], rts[ds(64, 64), :]
                )
                for qs in range(NQS):
                    op = psp.tile([128, 1024], F32, tag="avps")
                    for n2 in range(2):
                        nc.tensor.matmul(
                            op[:, ts(n2, 512)],
                            ot[:, ts(qs, 128)],
                            WO[:, ts(n2, 512)],
                            start=True,
                            stop=True,
                        )
                    osb = osbp.tile([128, D], F32, tag="osb")
                    if qs % 2 == 0:
                        nc.vector.tensor_copy(osb[:], op[:])
                    else:
                        nc.scalar.copy(osb[:], op[:])
                    nc.sync.dma_start(out[ds(q0 + qs * 128, 128), :], osb[:])


def build_nc(seq=SEQ, qcw=1024):
    assert seq % 512 == 0 and qcw % 512 == 0 and seq % qcw == 0
    assert (seq // 128) % 4 == 0
    nc = bacc.Bacc("TRN2", target_bir_lowering=False, debug=False,
                   num_devices=NCORES)
    with tile.TileContext(nc) as tc:
        _emit(nc, tc, seq, qcw)
    nc.compile()
    return nc


def shard_inputs(inputs, wq, bq, wk, bk, wv, bv, wo, seq=SEQ):
    X2 = np.asarray(inputs, np.float32).reshape(B * seq, D)
    XT = np.ascontiguousarray(X2.T)
    wq, wk, wv = (np.asarray(a, np.float32) for a in (wq, wk, wv))
    bq, bk, bv = (np.asarray(a, np.float32) for a in (bq, bk, bv))
    wo = np.asarray(wo, np.float32)
    in_maps = []
    for c in range(NCORES):
        sl = slice(c * HD, (c + 1) * HD)
        in_maps.append({
            "xt": XT,
            "wq": np.ascontiguousarray(wq[:, sl]),
            "wk": np.ascontiguousarray(wk[:, sl]),
            "wv": np.ascontiguousarray(wv[:, sl]),
            "bq": np.ascontiguousarray(bq[sl]).reshape(HD, 1),
            "bk": np.ascontiguousarray(bk[sl]).reshape(HD, 1),
            "bv": np.ascontiguousarray(bv[sl]).reshape(HD, 1),
            "wo": np.ascontiguousarray(wo[sl, :]),
        })
    return in_maps


_NC_CACHE = {}


def _get_nc(seq=SEQ, qcw=1024):
    key = (seq, qcw)
    if key not in _NC_CACHE:
        _NC_CACHE[key] = build_nc(seq, qcw)
    return _NC_CACHE[key]


def kernel(inputs, wq, bq, wk, bk, wv, bv, wo, bo):
    from concourse.bass_utils import run_bass_kernel_spmd

    nc = _get_nc()
    in_maps = shard_inputs(inputs, wq, bq, wk, bk, wv, bv, wo)
    res = run_bass_kernel_spmd(nc, in_maps, core_ids=list(range(NCORES)))
    acc = np.zeros((B * SEQ, D), np.float64)
    for c in range(NCORES):
        acc += res.results[c]["out"].astype(np.float64)
    acc += np.asarray(bo, np.float32).astype(np.float64)
    return acc.astype(np.float32).reshape(B, SEQ, D)
